# revision 2
# baseline (speedup 1.0000x reference)
"""Multi-head causal attention (B=4, S=2048, D=1024, H=16) on 8 TRN2 NeuronCores.

Sharding: core c handles batch b = c//2 and head-group g = c%2 (8 heads,
512 output channels). Wq/Wk/Wv column-split, Wo row-split; each core
returns a transposed partial output outT[e, s]; the host sums the two
partials per batch (the Wo row-split all-reduce done at gather time).

Per-core kernel (all matmuls fp32r unless noted):
  phase A: KT[dl, s] and V[s, dl] projections (V stored per-head with a
           ones column appended -> PV matmul emits softmax denominators
           for free).
  phase B (per 512-wide query chunk j):
    QT[dl, s] projection for the chunk;
    per head: scoresT tiles [128 keys, 512 q] = KT_h^T @ QT_h on PE,
    exp on ScalarE (scale=1/sqrt(dk) folded in; no max-subtraction --
    scores are ~N(0,1) so exp cannot overflow), bf16 expS, 0/1 mask
    multiply on partial tiles only (full upper-triangle tiles skipped
    entirely), PV accumulation [ctx|denom] in PSUM, normalize via
    reciprocal + ones-matmul partition-broadcast;
    out-projection: K=64 matmuls vs streamed WoT tiles -> outT[e, s].

The causal structure is derived from the actual `mask` input at run
time (any 0/1 mask works; tril and all-ones are the fast cases).
"""

import sys

sys.path.insert(0, "/opt/trn_rl_repo")

import numpy as np
import ml_dtypes

import concourse.bacc as bacc
import concourse.tile as tile
import concourse.mybir as mybir
from concourse.bass_utils import run_bass_kernel_spmd

B, S, D, H = 4, 2048, 1024, 16
DK = D // H          # 64
NCORES = 8
HG = 2               # head groups (tensor-parallel ways)
HL = H // HG         # 8 heads per core
DL = D // HG         # 512 local channels
NJ = S // 512        # 4 query chunks
NKT = S // 128       # 16 key tiles
NC8 = S // 256       # 8 x-chunks for projections
SCALE = 1.0 / float(np.sqrt(DK))

F32 = mybir.dt.float32
F32R = mybir.dt.float32r
BF16 = mybir.dt.bfloat16
F16 = mybir.dt.float16
EXP = mybir.ActivationFunctionType.Exp
MULT = mybir.AluOpType.mult


def _classify_mask(mask):
    """Per (q-chunk j, key-tile kt) classify the mask block.

    Returns (schedule, patterns):
      schedule[j] = list of (kt, pat_idx|None) -- None means all-valid;
      patterns = [NP, 128, 512] bf16 array of 0/1 tiles (NP >= 1).
    """
    m2 = np.asarray(mask).reshape(S, S) != 0
    schedule = []
    patterns = []
    pat_index = {}
    for j in range(NJ):
        row = []
        for kt in range(NKT):
            sub = m2[j * 512:(j + 1) * 512, kt * 128:(kt + 1) * 128]
            if not sub.any():
                continue
            if sub.all():
                row.append((kt, None))
                continue
            pat = np.ascontiguousarray(sub.T)  # [128 keys, 512 q]
            key = pat.tobytes()
            if key not in pat_index:
                pat_index[key] = len(patterns)
                patterns.append(pat)
            row.append((kt, pat_index[key]))
        schedule.append(row)
    if not patterns:
        patterns.append(np.ones((128, 512), bool))
    pats = np.stack(patterns).astype(ml_dtypes.bfloat16)
    return schedule, pats


def _build(schedule, npat):
    nc = bacc.Bacc("TRN2", target_bir_lowering=False, debug=False,
                   num_devices=NCORES)

    xqT = nc.dram_tensor("xqT", [D, S], F32R, kind="ExternalInput").ap()
    xkT = nc.dram_tensor("xkT", [D, S], F32R, kind="ExternalInput").ap()
    xvT = nc.dram_tensor("xvT", [D, S], F32R, kind="ExternalInput").ap()
    wqT = nc.dram_tensor("wqT", [D, DL], F32R, kind="ExternalInput").ap()
    wkT = nc.dram_tensor("wkT", [D, DL], F32R, kind="ExternalInput").ap()
    wvT = nc.dram_tensor("wvT", [D, DL], F32R, kind="ExternalInput").ap()
    # woT[h, d, e]: rows of Wo for head h's 64 local channels
    woT = nc.dram_tensor("woT", [HL, DK, D], F32R, kind="ExternalInput").ap()
    mpat = nc.dram_tensor("mpat", [npat, 128, 512], BF16,
                          kind="ExternalInput").ap()
    outT = nc.dram_tensor("outT", [D, S], F32, kind="ExternalOutput").ap()

    with tile.TileContext(nc) as tc:
        with (
            tc.tile_pool(name="res", bufs=1) as res,
            tc.tile_pool(name="wkv", bufs=2) as wkvp,
            tc.tile_pool(name="xin", bufs=4) as xin,
            tc.tile_pool(name="qt", bufs=1) as qtp,
            tc.tile_pool(name="ctx", bufs=1) as ctxp,
            tc.tile_pool(name="es", bufs=3) as esp,
            tc.tile_pool(name="wop", bufs=3) as wop,
            tc.tile_pool(name="outsb", bufs=2) as outsbp,
            tc.tile_pool(name="misc", bufs=3) as miscp,
            tc.tile_pool(name="pp", bufs=2, space="PSUM") as pp,
            tc.tile_pool(name="pscore", bufs=2, space="PSUM") as psc,
            tc.tile_pool(name="pctx", bufs=2, space="PSUM") as pcx,
        ):
            # ---- resident tensors
            wq_sb = res.tile([128, 8, DL], F32R, tag="wq")
            nc.sync.dma_start(wq_sb[:], wqT.rearrange("(t p) m -> p t m", p=128))
            wk_sb = wkvp.tile([128, 8, DL], F32R, tag="w")
            nc.sync.dma_start(wk_sb[:], wkT.rearrange("(t p) m -> p t m", p=128))
            wv_sb = wkvp.tile([128, 8, DL], F32R, tag="w")
            nc.sync.dma_start(wv_sb[:], wvT.rearrange("(t p) m -> p t m", p=128))
            mask_sb = res.tile([128, npat, 512], BF16, tag="mask")
            nc.sync.dma_start(mask_sb[:], mpat.rearrange("n p s -> p n s"))
            kt_sb = res.tile([128, 4, S], F32R, tag="kt")
            v_sb = res.tile([128, NKT, HL, DK + 1], BF16, tag="v")
            nc.vector.memset(v_sb[:, :, :, DK], 1.0)
            ones16 = res.tile([1, 64], F16, tag="ones16")
            nc.vector.memset(ones16[:], 1.0)

            # ---- phase A: KT and V projections, per 256-wide s-chunk
            for c in range(NC8):
                sl = slice(c * 256, (c + 1) * 256)
                xk_sb = xin.tile([128, 8, 256], F32R, tag="x")
                nc.sync.dma_start(
                    xk_sb[:], xkT.rearrange("(t p) s -> p t s", p=128)[:, :, sl])
                xv_sb = xin.tile([128, 8, 256], F32R, tag="x")
                nc.sync.dma_start(
                    xv_sb[:], xvT.rearrange("(t p) s -> p t s", p=128)[:, :, sl])
                for m in range(4):
                    ps = pp.tile([128, 512], F32, tag="pp")
                    for dt in range(8):
                        nc.tensor.matmul(
                            ps[:, 0:256], wk_sb[:, dt, m * 128:(m + 1) * 128],
                            xk_sb[:, dt, :], start=(dt == 0), stop=(dt == 7))
                    nc.vector.tensor_copy(kt_sb[:, m, sl], ps[:, 0:256])
                for st in range(2):
                    ps = pp.tile([128, 512], F32, tag="pp")
                    for dt in range(8):
                        nc.tensor.matmul(
                            ps[:], xv_sb[:, dt, st * 128:(st + 1) * 128],
                            wv_sb[:, dt, :], start=(dt == 0), stop=(dt == 7))
                    nc.vector.tensor_copy(
                        v_sb[:, 2 * c + st, :, 0:DK],
                        ps.rearrange("p (h d) -> p h d", d=DK))

            # ---- phase B: per query chunk
            for j in range(NJ):
                qt_sb = qtp.tile([128, 4, 512], F32R, tag="qt")
                for half in range(2):
                    sl = slice(j * 512 + half * 256, j * 512 + (half + 1) * 256)
                    xq_sb = xin.tile([128, 8, 256], F32R, tag="x")
                    nc.sync.dma_start(
                        xq_sb[:],
                        xqT.rearrange("(t p) s -> p t s", p=128)[:, :, sl])
                    for m in range(4):
                        ps = pp.tile([128, 512], F32, tag="pp")
                        for dt in range(8):
                            nc.tensor.matmul(
                                ps[:, 0:256],
                                wq_sb[:, dt, m * 128:(m + 1) * 128],
                                xq_sb[:, dt, :], start=(dt == 0), stop=(dt == 7))
                        nc.vector.tensor_copy(
                            qt_sb[:, m, half * 256:(half + 1) * 256],
                            ps[:, 0:256])

                ctx_sb = ctxp.tile([128, HL, 512], F32R, tag="ctx")
                kts = schedule[j]
                for h in range(HL):
                    po, mt = (h % 2) * 64, h // 2
                    q_h = qt_sb[po:po + 64, mt, :]
                    pc = pcx.tile([128, 512], F32, tag="pctx")
                    nkts = len(kts)
                    for g0 in range(0, nkts, 2):
                        grp = kts[g0:g0 + 2]
                        ng = len(grp)
                        sp = psc.tile([128, 2, 512], F32, tag="psc")
                        for i, (kt, _pat) in enumerate(grp):
                            nc.tensor.matmul(
                                sp[:, i, :],
                                kt_sb[po:po + 64, mt, kt * 128:(kt + 1) * 128],
                                q_h, start=True, stop=True)
                        es = esp.tile([128, 2, 512], BF16, tag="es")
                        nc.scalar.activation(es[:, 0:ng, :], sp[:, 0:ng, :],
                                             EXP, scale=SCALE)
                        for i, (kt, pat) in enumerate(grp):
                            if pat is not None:
                                nc.vector.tensor_tensor(
                                    es[:, i, :], es[:, i, :],
                                    mask_sb[:, pat, :], MULT)
                        for i, (kt, _pat) in enumerate(grp):
                            nc.tensor.matmul(
                                pc[0:DK + 1, :], v_sb[:, kt, h, :], es[:, i, :],
                                start=(g0 + i == 0), stop=(g0 + i == nkts - 1))
                    recip = miscp.tile([1, 512], F32, tag="recip")
                    nc.vector.reciprocal(recip[:], pc[DK:DK + 1, :])
                    recip16 = miscp.tile([1, 512], F16, tag="recip16")
                    nc.vector.tensor_copy(recip16[:], recip[:])
                    pb = pp.tile([128, 512], F32, tag="pp")
                    nc.tensor.matmul(pb[0:64, :], ones16[:], recip16[:],
                                     start=True, stop=True)
                    bc_sb = miscp.tile([64, 512], F32, tag="bc")
                    nc.scalar.copy(bc_sb[:], pb[0:64, :])
                    nc.vector.tensor_tensor(ctx_sb[0:64, h, :], pc[0:64, :],
                                            bc_sb[:], MULT)

                # out-projection for this chunk (K=64 per head block)
                sl = slice(j * 512, (j + 1) * 512)
                for m in range(8):
                    wo_m = wop.tile([64, 8, 128], F32R, tag="wo")
                    nc.sync.dma_start(
                        wo_m[:],
                        woT[:, :, m * 128:(m + 1) * 128].rearrange(
                            "h d e -> d h e"))
                    ps = pp.tile([128, 512], F32, tag="pp")
                    for h in range(HL):
                        nc.tensor.matmul(
                            ps[:], wo_m[0:64, h, :], ctx_sb[0:64, h, :],
                            start=(h == 0), stop=(h == HL - 1))
                    o_sb = outsbp.tile([128, 512], F32, tag="osb")
                    nc.vector.tensor_copy(o_sb[:], ps[:])
                    nc.sync.dma_start(outT[m * 128:(m + 1) * 128, sl], o_sb[:])

    nc.compile()
    return nc


_CACHE = {}


def _get_nc(mask):
    schedule, pats = _classify_mask(mask)
    key = (tuple(tuple(r) for r in schedule), pats.tobytes())
    if key not in _CACHE:
        _CACHE[key] = (_build(schedule, pats.shape[0]), pats)
    return _CACHE[key]


def make_in_maps(q, k, v, Wq, Wk, Wv, Wo, pats):
    in_maps = []
    for c in range(NCORES):
        b, g = c // HG, c % HG
        gsl = slice(g * DL, (g + 1) * DL)
        in_maps.append(dict(
            xqT=np.ascontiguousarray(q[b].T),
            xkT=np.ascontiguousarray(k[b].T),
            xvT=np.ascontiguousarray(v[b].T),
            wqT=np.ascontiguousarray(Wq[gsl, :].T),
            wkT=np.ascontiguousarray(Wk[gsl, :].T),
            wvT=np.ascontiguousarray(Wv[gsl, :].T),
            woT=np.ascontiguousarray(Wo[:, gsl].T).reshape(HL, DK, D),
            mpat=pats,
        ))
    return in_maps


def gather_out(results):
    out = np.empty((B, S, D), np.float32)
    for b in range(B):
        out[b] = (results[HG * b]["outT"] + results[HG * b + 1]["outT"]).T
    return out


def kernel(q, k, v, Wq, Wk, Wv, Wo, mask):
    q = np.asarray(q, np.float32)
    k = np.asarray(k, np.float32)
    v = np.asarray(v, np.float32)
    Wq = np.asarray(Wq, np.float32)
    Wk = np.asarray(Wk, np.float32)
    Wv = np.asarray(Wv, np.float32)
    Wo = np.asarray(Wo, np.float32)

    nc, pats = _get_nc(mask)
    in_maps = make_in_maps(q, k, v, Wq, Wk, Wv, Wo, pats)
    results = run_bass_kernel_spmd(
        nc, in_maps, core_ids=list(range(NCORES))).results
    return gather_out(results)


# revision 6
# speedup vs baseline: 1.0317x; 1.0317x over previous
"""Multi-head causal attention (B=4, S=2048, D=1024, H=16) on 8 TRN2 NeuronCores.

Sharding: core c handles batch b = c//2 and head-group g = c%2 (8 heads,
512 output channels). Wq/Wk/Wv column-split, Wo row-split; each core
returns a transposed partial output outT[e, s]; the host sums the two
partials per batch (the Wo row-split all-reduce done at gather time).

Per-core kernel (all matmuls fp32r unless noted):
  phase A: KT[dl, s] and V[s, dl] projections (V stored per-head with a
           ones column appended -> PV matmul emits softmax denominators
           for free).
  phase B (per 512-wide query chunk j):
    QT[dl, s] projection for the chunk;
    per head: scoresT tiles [128 keys, 512 q] = KT_h^T @ QT_h on PE,
    exp on ScalarE (scale=1/sqrt(dk) folded in; no max-subtraction --
    scores are ~N(0,1) so exp cannot overflow), bf16 expS, 0/1 mask
    multiply on partial tiles only (full upper-triangle tiles skipped
    entirely), PV accumulation [ctx|denom] in PSUM, normalize via
    reciprocal + ones-matmul partition-broadcast;
    out-projection: K=64 matmuls vs streamed WoT tiles -> outT[e, s].

The causal structure is derived from the actual `mask` input at run
time (any 0/1 mask works; tril and all-ones are the fast cases).
"""

import sys

sys.path.insert(0, "/opt/trn_rl_repo")

import numpy as np
import ml_dtypes

import concourse.bacc as bacc
import concourse.tile as tile
import concourse.mybir as mybir
from concourse.bass_utils import run_bass_kernel_spmd

B, S, D, H = 4, 2048, 1024, 16
DK = D // H          # 64
NCORES = 8
HG = 2               # head groups (tensor-parallel ways)
HL = H // HG         # 8 heads per core
DL = D // HG         # 512 local channels
NJ = S // 512        # 4 query chunks
NKT = S // 128       # 16 key tiles
NC8 = S // 256       # 8 x-chunks for projections
SCALE = 1.0 / float(np.sqrt(DK))

F32 = mybir.dt.float32
F32R = mybir.dt.float32r
BF16 = mybir.dt.bfloat16
F16 = mybir.dt.float16
EXP = mybir.ActivationFunctionType.Exp
MULT = mybir.AluOpType.mult


def _classify_mask(mask):
    """Per (q-chunk j, key-tile kt) classify the mask block.

    Returns (schedule, patterns):
      schedule[j] = list of (kt, pat_idx|None) -- None means all-valid;
      patterns = [NP, 128, 512] bf16 array of 0/1 tiles (NP >= 1).
    """
    m2 = np.asarray(mask).reshape(S, S) != 0
    schedule = []
    patterns = []
    pat_index = {}
    for j in range(NJ):
        row = []
        for kt in range(NKT):
            sub = m2[j * 512:(j + 1) * 512, kt * 128:(kt + 1) * 128]
            if not sub.any():
                continue
            if sub.all():
                row.append((kt, None))
                continue
            pat = np.ascontiguousarray(sub.T)  # [128 keys, 512 q]
            key = pat.tobytes()
            if key not in pat_index:
                pat_index[key] = len(patterns)
                patterns.append(pat)
            row.append((kt, pat_index[key]))
        schedule.append(row)
    if not patterns:
        patterns.append(np.ones((128, 512), bool))
    pats = np.stack(patterns).astype(ml_dtypes.bfloat16)
    return schedule, pats


def _build(schedule, npat):
    nc = bacc.Bacc("TRN2", target_bir_lowering=False, debug=False,
                   num_devices=NCORES)

    xqT = nc.dram_tensor("xqT", [D, S], F32R, kind="ExternalInput").ap()
    xkT = nc.dram_tensor("xkT", [D, S], F32R, kind="ExternalInput").ap()
    xvT = nc.dram_tensor("xvT", [D, S], F32R, kind="ExternalInput").ap()
    wqT = nc.dram_tensor("wqT", [D, DL], F32R, kind="ExternalInput").ap()
    wkT = nc.dram_tensor("wkT", [D, DL], F32R, kind="ExternalInput").ap()
    wvT = nc.dram_tensor("wvT", [D, DL], F32R, kind="ExternalInput").ap()
    # woT[h, d, e]: rows of Wo for head h's 64 local channels
    woT = nc.dram_tensor("woT", [HL, DK, D], F32R, kind="ExternalInput").ap()
    mpat = nc.dram_tensor("mpat", [npat, 128, 512], BF16,
                          kind="ExternalInput").ap()
    outT = nc.dram_tensor("outT", [D, S], F32, kind="ExternalOutput").ap()

    with tile.TileContext(nc) as tc:
        with (
            tc.tile_pool(name="res", bufs=1) as res,
            tc.tile_pool(name="wkv", bufs=2) as wkvp,
            tc.tile_pool(name="xin", bufs=4) as xin,
            tc.tile_pool(name="qt", bufs=1) as qtp,
            tc.tile_pool(name="ctx", bufs=1) as ctxp,
            tc.tile_pool(name="es", bufs=3) as esp,
            tc.tile_pool(name="wop", bufs=3) as wop,
            tc.tile_pool(name="outsb", bufs=2) as outsbp,
            tc.tile_pool(name="misc", bufs=3) as miscp,
            tc.tile_pool(name="pp", bufs=2, space="PSUM") as pp,
            tc.tile_pool(name="pscore", bufs=2, space="PSUM") as psc,
            tc.tile_pool(name="pctx", bufs=2, space="PSUM") as pcx,
        ):
            # ---- resident tiles (DMAs emitted in order of first use)
            kt_sb = res.tile([128, 4, S], F32R, tag="kt")
            v_sb = res.tile([128, NKT, HL, DK + 1], BF16, tag="v")
            nc.vector.memset(v_sb[:, :, :, DK], 1.0)
            ones16 = res.tile([1, 64], F16, tag="ones16")
            nc.vector.memset(ones16[:], 1.0)

            xkc = [xin.tile([128, 8, 256], F32R, tag="x", name=f"xk{i}") for i in range(2)]
            nc.sync.dma_start(
                xkc[0][:], xkT.rearrange("(t p) s -> p t s", p=128)[:, :, 0:256])
            wk_sb = wkvp.tile([128, 8, DL], F32R, tag="w")
            for m in range(4):
                nc.sync.dma_start(
                    wk_sb[:, :, m * 128:(m + 1) * 128],
                    wkT.rearrange("(t p) m -> p t m", p=128)[
                        :, :, m * 128:(m + 1) * 128])
            xvc = [xin.tile([128, 8, 256], F32R, tag="x", name=f"xv{i}") for i in range(2)]
            nc.sync.dma_start(
                xvc[0][:], xvT.rearrange("(t p) s -> p t s", p=128)[:, :, 0:256])
            wv_sb = wkvp.tile([128, 8, DL], F32R, tag="w")
            nc.sync.dma_start(wv_sb[:], wvT.rearrange("(t p) m -> p t m", p=128))
            wq_sb = res.tile([128, 8, DL], F32R, tag="wq")
            nc.sync.dma_start(wq_sb[:], wqT.rearrange("(t p) m -> p t m", p=128))
            mask_sb = res.tile([128, npat, 512], BF16, tag="mask")
            nc.sync.dma_start(mask_sb[:], mpat.rearrange("n p s -> p n s"))

            # ---- phase A: KT and V projections, per 256-wide s-chunk
            for c in range(NC8):
                sl = slice(c * 256, (c + 1) * 256)
                xk_sb, xv_sb = xkc[c % 2], xvc[c % 2]
                if c + 1 < NC8:
                    nsl = slice((c + 1) * 256, (c + 2) * 256)
                    xkc[(c + 1) % 2] = xin.tile([128, 8, 256], F32R, tag="x", name=f"xk{c}")
                    nc.sync.dma_start(
                        xkc[(c + 1) % 2][:],
                        xkT.rearrange("(t p) s -> p t s", p=128)[:, :, nsl])
                    xvc[(c + 1) % 2] = xin.tile([128, 8, 256], F32R, tag="x", name=f"xv{c}")
                    nc.sync.dma_start(
                        xvc[(c + 1) % 2][:],
                        xvT.rearrange("(t p) s -> p t s", p=128)[:, :, nsl])
                for m in range(4):
                    ps = pp.tile([128, 512], F32, tag="pp")
                    for dt in range(8):
                        nc.tensor.matmul(
                            ps[:, 0:256], wk_sb[:, dt, m * 128:(m + 1) * 128],
                            xk_sb[:, dt, :], start=(dt == 0), stop=(dt == 7))
                    nc.vector.tensor_copy(kt_sb[:, m, sl], ps[:, 0:256])
                for st in range(2):
                    ps = pp.tile([128, 512], F32, tag="pp")
                    for dt in range(8):
                        nc.tensor.matmul(
                            ps[:], xv_sb[:, dt, st * 128:(st + 1) * 128],
                            wv_sb[:, dt, :], start=(dt == 0), stop=(dt == 7))
                    nc.vector.tensor_copy(
                        v_sb[:, 2 * c + st, :, 0:DK],
                        ps.rearrange("p (h d) -> p h d", d=DK))

            # ---- phase B: per query chunk, software-pipelined group stream
            prev_ctx = None  # (ctx_sb, j) pending out-projection
            for j in range(NJ):
                qt_sb = qtp.tile([128, 4, 512], F32R, tag="qt")
                for half in range(2):
                    sl = slice(j * 512 + half * 256, j * 512 + (half + 1) * 256)
                    xq_sb = xin.tile([128, 8, 256], F32R, tag="x")
                    nc.sync.dma_start(
                        xq_sb[:],
                        xqT.rearrange("(t p) s -> p t s", p=128)[:, :, sl])
                    for m in range(4):
                        ps = pp.tile([128, 512], F32, tag="pp")
                        for dt in range(8):
                            nc.tensor.matmul(
                                ps[:, 0:256],
                                wq_sb[:, dt, m * 128:(m + 1) * 128],
                                xq_sb[:, dt, :], start=(dt == 0), stop=(dt == 7))
                        nc.vector.tensor_copy(
                            qt_sb[:, m, half * 256:(half + 1) * 256],
                            ps[:, 0:256])

                # out-projection of the previous chunk overlaps this one
                if prev_ctx is not None:
                    _emit_outproj(nc, wop, pp, outsbp, woT, outT, *prev_ctx)
                    prev_ctx = None

                ctx_sb = ctxp.tile([128, HL, 512], F32R, tag="ctx")
                kts = schedule[j]
                nkts = len(kts)
                # flat group stream: (h, g0) with 1-group PV lag and
                # deferred normalize chains
                groups = [(h, g0) for h in range(HL)
                          for g0 in range(0, nkts, 2)]
                es_tiles = {}
                pc_tiles = {}
                pending_pv = None
                chain2 = []  # [(due_idx, h, pc, recip16)]
                for gi, (h, g0) in enumerate(groups):
                    po, mt = (h % 2) * 64, h // 2
                    grp = kts[g0:g0 + 2]
                    ng = len(grp)
                    if g0 == 0:
                        pc_tiles[h] = pcx.tile([128, 512], F32, tag="pctx", name=f"pc{h}")
                    sp = psc.tile([128, 2, 512], F32, tag="psc")
                    for i, (kt, _pat) in enumerate(grp):
                        nc.tensor.matmul(
                            sp[:, i, :],
                            kt_sb[po:po + 64, mt, kt * 128:(kt + 1) * 128],
                            qt_sb[po:po + 64, mt, :], start=True, stop=True)
                    es = esp.tile([128, 2, 512], BF16, tag="es")
                    nc.scalar.activation(es[:, 0:ng, :], sp[:, 0:ng, :],
                                         EXP, scale=SCALE)
                    for i, (kt, pat) in enumerate(grp):
                        if pat is not None:
                            nc.vector.tensor_tensor(
                                es[:, i, :], es[:, i, :],
                                mask_sb[:, pat, :], MULT)
                    es_tiles[(h, g0)] = es
                    # deferred chain part 2 (bcast matmul + normalize)
                    for due, hh, pcc, r16 in list(chain2):
                        if gi >= due:
                            _emit_chain2(nc, pp, miscp, ones16, ctx_sb, hh,
                                         pcc, r16)
                            chain2.remove((due, hh, pcc, r16))
                    # PV lags one group
                    if pending_pv is not None:
                        _emit_pv(nc, v_sb, es_tiles, pc_tiles, kts,
                                 pending_pv)
                        ph, pg0 = pending_pv
                        if pg0 + 2 >= nkts:  # head ph finished
                            r16 = _emit_chain1(nc, miscp, pc_tiles[ph])
                            chain2.append((gi + 2, ph, pc_tiles[ph], r16))
                    pending_pv = (h, g0)
                # flush
                if pending_pv is not None:
                    _emit_pv(nc, v_sb, es_tiles, pc_tiles, kts, pending_pv)
                    ph, pg0 = pending_pv
                    r16 = _emit_chain1(nc, miscp, pc_tiles[ph])
                    chain2.append((0, ph, pc_tiles[ph], r16))
                for due, hh, pcc, r16 in chain2:
                    _emit_chain2(nc, pp, miscp, ones16, ctx_sb, hh, pcc, r16)
                prev_ctx = (ctx_sb, j)

            _emit_outproj(nc, wop, pp, outsbp, woT, outT, *prev_ctx)

    nc.compile()
    return nc


def _emit_pv(nc, v_sb, es_tiles, pc_tiles, kts, key):
    h, g0 = key
    nkts = len(kts)
    grp = kts[g0:g0 + 2]
    es = es_tiles.pop(key)
    pc = pc_tiles[h]
    for i, (kt, _pat) in enumerate(grp):
        nc.tensor.matmul(
            pc[0:DK + 1, :], v_sb[:, kt, h, :], es[:, i, :],
            start=(g0 + i == 0), stop=(g0 + i == nkts - 1))


def _emit_chain1(nc, miscp, pc):
    recip = miscp.tile([1, 512], F32, tag="recip")
    nc.vector.reciprocal(recip[:], pc[DK:DK + 1, :])
    recip16 = miscp.tile([1, 512], F16, tag="recip16")
    nc.vector.tensor_copy(recip16[:], recip[:])
    return recip16


def _emit_chain2(nc, pp, miscp, ones16, ctx_sb, h, pc, recip16):
    pb = pp.tile([128, 512], F32, tag="pp")
    nc.tensor.matmul(pb[0:64, :], ones16[:], recip16[:], start=True, stop=True)
    bc_sb = miscp.tile([64, 512], F32, tag="bc")
    nc.scalar.copy(bc_sb[:], pb[0:64, :])
    nc.vector.tensor_tensor(ctx_sb[0:64, h, :], pc[0:64, :], bc_sb[:], MULT)


def _emit_outproj(nc, wop, pp, outsbp, woT, outT, ctx_sb, j):
    sl = slice(j * 512, (j + 1) * 512)
    for m in range(8):
        wo_m = wop.tile([64, 8, 128], F32R, tag="wo")
        nc.sync.dma_start(
            wo_m[:],
            woT[:, :, m * 128:(m + 1) * 128].rearrange("h d e -> d h e"))
        ps = pp.tile([128, 512], F32, tag="pp")
        for h in range(HL):
            nc.tensor.matmul(
                ps[:], wo_m[0:64, h, :], ctx_sb[0:64, h, :],
                start=(h == 0), stop=(h == HL - 1))
        o_sb = outsbp.tile([128, 512], F32, tag="osb")
        nc.vector.tensor_copy(o_sb[:], ps[:])
        nc.sync.dma_start(outT[m * 128:(m + 1) * 128, sl], o_sb[:])


_CACHE = {}


def _get_nc(mask):
    schedule, pats = _classify_mask(mask)
    key = (tuple(tuple(r) for r in schedule), pats.tobytes())
    if key not in _CACHE:
        _CACHE[key] = (_build(schedule, pats.shape[0]), pats)
    return _CACHE[key]


def make_in_maps(q, k, v, Wq, Wk, Wv, Wo, pats):
    in_maps = []
    for c in range(NCORES):
        b, g = c // HG, c % HG
        gsl = slice(g * DL, (g + 1) * DL)
        in_maps.append(dict(
            xqT=np.ascontiguousarray(q[b].T),
            xkT=np.ascontiguousarray(k[b].T),
            xvT=np.ascontiguousarray(v[b].T),
            wqT=np.ascontiguousarray(Wq[gsl, :].T),
            wkT=np.ascontiguousarray(Wk[gsl, :].T),
            wvT=np.ascontiguousarray(Wv[gsl, :].T),
            woT=np.ascontiguousarray(Wo[:, gsl].T).reshape(HL, DK, D),
            mpat=pats,
        ))
    return in_maps


def gather_out(results):
    out = np.empty((B, S, D), np.float32)
    for b in range(B):
        out[b] = (results[HG * b]["outT"] + results[HG * b + 1]["outT"]).T
    return out


def kernel(q, k, v, Wq, Wk, Wv, Wo, mask):
    q = np.asarray(q, np.float32)
    k = np.asarray(k, np.float32)
    v = np.asarray(v, np.float32)
    Wq = np.asarray(Wq, np.float32)
    Wk = np.asarray(Wk, np.float32)
    Wv = np.asarray(Wv, np.float32)
    Wo = np.asarray(Wo, np.float32)

    nc, pats = _get_nc(mask)
    in_maps = make_in_maps(q, k, v, Wq, Wk, Wv, Wo, pats)
    results = run_bass_kernel_spmd(
        nc, in_maps, core_ids=list(range(NCORES))).results
    return gather_out(results)
